# revision 10
# baseline (speedup 1.0000x reference)
"""Trainium2 Bass kernel for the two-level Haar-DWT detail (L1) loss.

Strategy (pure data parallel over batch, 8 NeuronCores):
  - Host casts both inputs to bf16 and permutes each image row's 512
    columns into mod-4 parity groups [c%4==0 | c%4==2 | c%4==1 | c%4==3],
    so the level-1 even/odd halves AND the level-2 parity quarters are
    all contiguous slices on chip.  Per-core data is laid out as
    [128, 48*512] per tensor (partition p = row p of each 128-row
    block), giving the DMA 1.5 MiB transfers with 12 KiB contiguous
    per-partition runs.
  - The full 12.6 MB/core bf16 stream is HBM-bound (~358 GB/s/core);
    compute is arranged so every engine stays under that ~35 us:
      * o-chunks stream on the Sync HWDGE queue, t-chunks on the GpSimd
        SWDGE queue (two queues overlap; the ACT queue stays clean).
      * DVE: d = o - t, then level-1 column combines cs/cd, as three
        packed-bf16 tensor_tensor ops per 12-block group (2x mode).
      * PE: all row combines, in self-contained 3-block psum units
        [S(b,b+1) | D(b,b+1) | S(b+2) | D(b+2) | L2(3 blocks)] =
        [128, 1920] so a unit never crosses a psum bank inside one
        matmul; two units in flight (double buffered).
      * ScalarE: ONE fused Abs-activation + accum_out per unit
        (immediate zero bias to skip the SBUF const read).
  - All band values share one global 1/(4*n1) divisor (LL1's 0.1 and
    level-2's 0.5 weights are baked into the matmul weights), so the
    per-partition accumulator columns are just summed at the end; the
    host combines the 8 [128,4] results in float64.
"""

import numpy as np

B, C, H, W = 32, 3, 512, 512
N_CORES = 8
B_PER_CORE = B // N_CORES
ROWS = B_PER_CORE * C * H          # 6144 image rows per core
COLS = W                           # 512
NBLK = ROWS // 128                 # 48 blocks of [128, 512]
TCOLS = NBLK * COLS                # 24576 cols per tensor in SBUF layout
OT_COLS = 2 * TCOLS                # o | t side by side

G_BLK = 12                         # blocks per DMA/DVE group
N_G = NBLK // G_BLK                # 4 groups
U_BLK = 3                          # blocks per psum unit
N_U = NBLK // U_BLK                # 16 units

_CACHE = {}


def _make_weights():
    import ml_dtypes
    q = ml_dtypes.bfloat16(0.1)  # LL1 loss weight, baked into W1q
    # w1q[k, m]: row pair-combine for the S (col-sum) path.
    # m<64: +q at rows 2m, 2m+1 (pair sum -> LL1, pre-weighted);
    # m=64+mm: -1/+1 (pair diff -> LH1).
    w1q = np.zeros((128, 128), ml_dtypes.bfloat16)
    w1 = np.zeros((128, 128), ml_dtypes.bfloat16)
    for m in range(64):
        w1q[2 * m, m] = q
        w1q[2 * m + 1, m] = q
        w1q[2 * m, 64 + m] = -1.0
        w1q[2 * m + 1, 64 + m] = 1.0
        # plain +-1 for the D (col-diff) path: HL1 | HH1
        w1[2 * m, m] = 1.0
        w1[2 * m + 1, m] = 1.0
        w1[2 * m, 64 + m] = -1.0
        w1[2 * m + 1, 64 + m] = 1.0
    # Level 2 in two accumulating matmuls: psum2 = w2neg @ cs_e +
    # w2pos @ cs_o (cs_e/cs_o = even/odd level-1 column pairs, stored
    # contiguously by the host's parity permutation).  Rows: [HH2 (diff
    # of cd2); HL2 (sum of cd2); LH2 (diff of cs2); 0], with
    # cd2 = cs_o - cs_e, cs2 = cs_e + cs_o.  Zero-padded to 128 outputs
    # so psum rows 96:128 are exact zeros.
    w2neg = np.zeros((128, 128), ml_dtypes.bfloat16)
    w2pos = np.zeros((128, 128), ml_dtypes.bfloat16)
    for m in range(32):
        for r in range(4):
            sd = -1.0 if r < 2 else 1.0  # 4-row diff pattern
            row = 4 * m + r
            w2neg[row, m] = -sd          # HH2
            w2pos[row, m] = sd
            w2neg[row, 32 + m] = -1.0    # HL2
            w2pos[row, 32 + m] = 1.0
            w2neg[row, 64 + m] = sd      # LH2
            w2pos[row, 64 + m] = sd
    return w1q, w1, w2neg, w2pos


def _build_bass():
    from contextlib import ExitStack

    import concourse.bacc as bacc
    import concourse.bass as bass
    import concourse.mybir as mybir
    import concourse.tile as tile

    F32 = mybir.dt.float32
    BF16 = mybir.dt.bfloat16
    X = mybir.AxisListType.X
    ADD = mybir.AluOpType.add
    ABS = mybir.ActivationFunctionType.Abs

    nc = bacc.Bacc("TRN2", target_bir_lowering=False, debug=False,
                   num_devices=N_CORES)
    ot_d = nc.dram_tensor("ot", [128, OT_COLS], BF16,
                          kind="ExternalInput").ap()
    w_d = nc.dram_tensor("w", [128, 512], BF16, kind="ExternalInput").ap()
    res_d = nc.dram_tensor("res", [128, 4], F32, kind="ExternalOutput").ap()

    with tile.TileContext(nc) as tc, ExitStack() as ctx:
        consts = ctx.enter_context(tc.tile_pool(name="consts", bufs=1))
        dpool = ctx.enter_context(tc.tile_pool(name="dpool", bufs=2))
        cpool = ctx.enter_context(tc.tile_pool(name="cpool", bufs=2))
        psU = ctx.enter_context(tc.tile_pool(name="psU", bufs=2,
                                             space="PSUM"))

        ot_sb = consts.tile([128, OT_COLS], BF16)
        w_all = consts.tile([128, 512], BF16)
        w1q_t = w_all[:, 0:128]
        w1_t = w_all[:, 128:256]
        w2n_t = w_all[:, 256:384]
        w2p_t = w_all[:, 384:512]

        acc = consts.tile([128, 32], F32)
        res_t = consts.tile([128, 4], F32)
        nc.vector.memset(acc[:], 0.0)
        nc.vector.memset(res_t[:], 0.0)

        mm = nc.tensor.matmul

        # Warm the PE to full p-state during the framework preamble.
        warm = consts.tile([128, 512], BF16)
        nc.vector.memset(warm[:], 0.0)
        pswarm = psU.tile([128, 1920], F32, tag="u")
        for _ in range(6):
            mm(pswarm[:, 0:512], lhsT=warm[:, 0:128], rhs=warm[:],
               start=True, stop=True)

        def act_abs(out, in_, accum_out):
            # Abs activation with an IMMEDIATE zero bias (bass converts
            # float biases of non-Copy funcs to an SBUF const AP, which
            # costs ~185 ns extra init per activation).
            imm = lambda v: mybir.ImmediateValue(dtype=F32, value=v)
            eng = nc.scalar
            return eng.add_instruction(mybir.InstActivation(
                name=nc.get_next_instruction_name(),
                func=ABS,
                ins=[eng.lower_ap(in_), imm(0.0), imm(1.0), imm(0.0)],
                outs=[eng.lower_ap(out), eng.lower_ap(accum_out)]))

        # ---- DMA: o-chunks on Sync HWDGE, t-chunks on GpSimd SWDGE
        # (keeps the ACT queue free), weights first on GpSimd.
        nc.gpsimd.dma_start(w_all[:], w_d)
        for g in range(N_G):
            c0 = g * G_BLK * COLS
            cw = G_BLK * COLS
            src_o = bass.AP(tensor=ot_d.tensor, offset=c0,
                            ap=[[OT_COLS, 128], [1, cw]])
            src_t = bass.AP(tensor=ot_d.tensor, offset=TCOLS + c0,
                            ap=[[OT_COLS, 128], [1, cw]])
            nc.sync.dma_start(ot_sb[:, c0:c0 + cw], src_o)
            nc.gpsimd.dma_start(ot_sb[:, TCOLS + c0:TCOLS + c0 + cw], src_t)

        csd_tiles = [None] * N_G
        n_acc = [0]

        def next_acc():
            c = n_acc[0]
            n_acc[0] += 1
            return acc[:, c:c + 1]

        def emit_dve_group(g):
            c0 = g * G_BLK * COLS
            cw = G_BLK * COLS
            d = dpool.tile([128, cw], BF16, tag="d")
            d3 = d[:].rearrange("p (b c) -> p b c", b=G_BLK)
            o3 = ot_sb[:, c0:c0 + cw].rearrange("p (b c) -> p b c", b=G_BLK)
            t3 = ot_sb[:, TCOLS + c0:TCOLS + c0 + cw].rearrange(
                "p (b c) -> p b c", b=G_BLK)
            nc.vector.tensor_sub(d3, o3, t3)

            csd = cpool.tile([128, cw], BF16, tag="csd")
            half = cw // 2
            cs3 = csd[:, 0:half].rearrange("p (b c) -> p b c", b=G_BLK)
            cd3 = csd[:, half:cw].rearrange("p (b c) -> p b c", b=G_BLK)
            de = d3[:, :, 0:COLS // 2]
            do = d3[:, :, COLS // 2:COLS]
            nc.vector.tensor_add(cs3, de, do)
            nc.vector.tensor_sub(cd3, do, de)
            csd_tiles[g] = csd

        def cs_view(b, nb):
            # cs cols for blocks [b, b+nb): [128, 256*nb] contiguous
            g, j = b // G_BLK, b % G_BLK
            return csd_tiles[g][:, 256 * j:256 * (j + nb)]

        def cd_view(b, nb):
            g, j = b // G_BLK, b % G_BLK
            off = G_BLK * 256
            return csd_tiles[g][:, off + 256 * j:off + 256 * (j + nb)]

        def cs_parity(b, nb, odd):
            # cs_e / cs_o over blocks [b, b+nb): [128, nb, 128]
            v = cs_view(b, nb).rearrange("p (b c) -> p b c", b=nb)
            return v[:, :, 128:256] if odd else v[:, :, 0:128]

        for u in range(N_U):
            b0 = u * U_BLK
            g = b0 // G_BLK
            if csd_tiles[g] is None:
                emit_dve_group(g)

            # unit psum [128, 1920]:
            # [S(b0,b0+1) 512 | D(b0,b0+1) 512 | S(b0+2) 256 |
            #  D(b0+2) 256 | L2(b0..b0+2) 384] -- no matmul output
            # crosses a psum bank boundary.
            uT = psU.tile([128, 1920], F32, tag="u")
            mm(uT[:, 0:512], lhsT=w1q_t, rhs=cs_view(b0, 2),
               start=True, stop=True)
            mm(uT[:, 1024:1280], lhsT=w1q_t, rhs=cs_view(b0 + 2, 1),
               start=True, stop=True)
            mm(uT[:, 512:1024], lhsT=w1_t, rhs=cd_view(b0, 2),
               start=True, stop=True)
            mm(uT[:, 1280:1536], lhsT=w1_t, rhs=cd_view(b0 + 2, 1),
               start=True, stop=True)
            mm(uT[:, 1536:1920], lhsT=w2n_t,
               rhs=cs_parity(b0, 3, odd=False), start=True, stop=False)
            mm(uT[:, 1536:1920], lhsT=w2p_t,
               rhs=cs_parity(b0, 3, odd=True), start=False, stop=True)

            act_abs(uT[:], uT[:], next_acc())

        res_col = res_t[:, 0:1]
        nc.vector.tensor_reduce(res_col, acc[:, 0:n_acc[0]], axis=X, op=ADD)
        nc.sync.dma_start(res_d, res_t[:])

    nc.compile()
    return nc


def _get_bass():
    if "nc" not in _CACHE:
        _CACHE["nc"] = _build_bass()
    return _CACHE["nc"]


def _numpy_reference(output, target):
    """Full-precision fallback (only for the never-hit mixed-normalize case)."""
    o = output.astype(np.float64)
    t = target.astype(np.float64)
    if o.min() < 0:
        o = (o + 1.0) * 0.5
    if t.min() < 0:
        t = (t + 1.0) * 0.5

    def dwt(x):
        a = x[:, :, 0::2, 0::2]
        b = x[:, :, 0::2, 1::2]
        c = x[:, :, 1::2, 0::2]
        d = x[:, :, 1::2, 1::2]
        return (0.5 * (a + b + c + d), 0.5 * (-a - b + c + d),
                0.5 * (-a + b - c + d), 0.5 * (a - b - c + d))

    ll_o, lh_o, hl_o, hh_o = dwt(o)
    ll_t, lh_t, hl_t, hh_t = dwt(t)
    tot = (np.abs(lh_o - lh_t).mean() + np.abs(hl_o - hl_t).mean()
           + np.abs(hh_o - hh_t).mean() + 0.1 * np.abs(ll_o - ll_t).mean())
    _, lh2_o, hl2_o, hh2_o = dwt(ll_o)
    _, lh2_t, hl2_t, hh2_t = dwt(ll_t)
    tot += 0.5 * (np.abs(lh2_o - lh2_t).mean() + np.abs(hl2_o - hl2_t).mean()
                  + np.abs(hh2_o - hh2_t).mean())
    return np.float32(tot)


_PERM = np.concatenate([np.arange(0, 512, 4), np.arange(2, 512, 4),
                        np.arange(1, 512, 4), np.arange(3, 512, 4)])


def _run_device(o, t, trace=False):
    """Shard [32,3,512,512] f32 arrays over 8 cores and run the Bass NEFF."""
    import ml_dtypes
    from concourse.bass_utils import run_bass_kernel_spmd

    nc = _get_bass()
    wcat = np.concatenate(_make_weights(), axis=1)

    in_maps = []
    for c in range(N_CORES):
        sl = slice(c * B_PER_CORE, (c + 1) * B_PER_CORE)
        otb = np.empty((128, OT_COLS), ml_dtypes.bfloat16)
        for i, x in enumerate((o, t)):
            xb = x[sl].reshape(ROWS, COLS)[:, _PERM].astype(ml_dtypes.bfloat16)
            otb[:, i * TCOLS:(i + 1) * TCOLS] = (
                xb.reshape(NBLK, 128, COLS).transpose(1, 0, 2)
                .reshape(128, TCOLS))
        in_maps.append({"ot": otb, "w": wcat})
    res = run_bass_kernel_spmd(nc, in_maps, core_ids=list(range(N_CORES)),
                               trace=trace)
    _CACHE["last_result"] = res
    return res


def combine(results, both_norm=True):
    """Combine per-core [128, 4] abs-sum tensors into the scalar loss."""
    m = 0.0
    for r in results:
        m += r.astype(np.float64)[:, 0].sum()
    n1 = float(B * C * (H // 2) * (W // 2))
    scale = 4.0 * n1 if both_norm else 2.0 * n1
    return np.float32(m / scale)


def kernel(output, target):
    o = np.ascontiguousarray(np.asarray(output, dtype=np.float32))
    t = np.ascontiguousarray(np.asarray(target, dtype=np.float32))
    o_norm = bool(o.min() < 0.0)
    t_norm = bool(t.min() < 0.0)
    if o_norm != t_norm:
        # Normalization applied to only one input: the difference is no
        # longer a pure scale of o - t.  Practically unreachable for the
        # randn inputs this problem uses.
        return _numpy_reference(o, t)

    results = [r["res"] for r in _run_device(o, t).results]
    return combine(results, both_norm=o_norm)


# revision 11
# speedup vs baseline: 1.1892x; 1.1892x over previous
"""Trainium2 Bass kernel for the two-level Haar-DWT detail (L1) loss.

Strategy (pure data parallel over batch, 8 NeuronCores):
  - Host casts both inputs to bf16 and permutes each image row's 512
    columns into mod-4 parity groups [c%4==0 | c%4==2 | c%4==1 | c%4==3],
    so the level-1 even/odd halves AND the level-2 parity quarters are
    all contiguous slices on chip.  Per-core data is laid out as
    [128, 48*512] per tensor (partition p = row p of each 128-row
    block), giving the DMA 1.5 MiB transfers with 12 KiB contiguous
    per-partition runs.
  - The full 12.6 MB/core bf16 stream is HBM-bound (~358 GB/s/core);
    compute is arranged so every engine stays under that ~35 us:
      * o-chunks stream on the Sync HWDGE queue, t-chunks on the GpSimd
        SWDGE queue (two queues overlap; the ACT queue stays clean).
      * DVE: d = o - t, then level-1 column combines cs/cd, as three
        packed-bf16 tensor_tensor ops per 12-block group (2x mode).
      * PE: all row combines, in self-contained 3-block psum units
        [S(b,b+1) | D(b,b+1) | S(b+2) | D(b+2) | L2(3 blocks)] =
        [128, 1920] so a unit never crosses a psum bank inside one
        matmul; two units in flight (double buffered).
      * ScalarE: ONE fused Abs-activation + accum_out per unit
        (immediate zero bias to skip the SBUF const read).
  - All band values share one global 1/(4*n1) divisor (LL1's 0.1 and
    level-2's 0.5 weights are baked into the matmul weights), so the
    per-partition accumulator columns are just summed at the end; the
    host combines the 8 [128,4] results in float64.
"""

import numpy as np

B, C, H, W = 32, 3, 512, 512
N_CORES = 8
B_PER_CORE = B // N_CORES
ROWS = B_PER_CORE * C * H          # 6144 image rows per core
COLS = W                           # 512
NBLK = ROWS // 128                 # 48 blocks of [128, 512]
TCOLS = NBLK * COLS                # 24576 cols per tensor in SBUF layout
OT_COLS = 2 * TCOLS                # o | t side by side

G_BLK = 12                         # blocks per DMA/DVE group
N_G = NBLK // G_BLK                # 4 groups
U_BLK = 3                          # blocks per psum unit
N_U = NBLK // U_BLK                # 16 units

_CACHE = {}


def _make_weights():
    import ml_dtypes
    q = ml_dtypes.bfloat16(0.1)  # LL1 loss weight, baked into W1q
    # w1q[k, m]: row pair-combine for the S (col-sum) path.
    # m<64: +q at rows 2m, 2m+1 (pair sum -> LL1, pre-weighted);
    # m=64+mm: -1/+1 (pair diff -> LH1).
    w1q = np.zeros((128, 128), ml_dtypes.bfloat16)
    w1 = np.zeros((128, 128), ml_dtypes.bfloat16)
    for m in range(64):
        w1q[2 * m, m] = q
        w1q[2 * m + 1, m] = q
        w1q[2 * m, 64 + m] = -1.0
        w1q[2 * m + 1, 64 + m] = 1.0
        # plain +-1 for the D (col-diff) path: HL1 | HH1
        w1[2 * m, m] = 1.0
        w1[2 * m + 1, m] = 1.0
        w1[2 * m, 64 + m] = -1.0
        w1[2 * m + 1, 64 + m] = 1.0
    # Level 2 in two accumulating matmuls: psum2 = w2neg @ cs_e +
    # w2pos @ cs_o (cs_e/cs_o = even/odd level-1 column pairs, stored
    # contiguously by the host's parity permutation).  Rows: [HH2 (diff
    # of cd2); HL2 (sum of cd2); LH2 (diff of cs2); 0], with
    # cd2 = cs_o - cs_e, cs2 = cs_e + cs_o.  Zero-padded to 128 outputs
    # so psum rows 96:128 are exact zeros.
    w2neg = np.zeros((128, 128), ml_dtypes.bfloat16)
    w2pos = np.zeros((128, 128), ml_dtypes.bfloat16)
    for m in range(32):
        for r in range(4):
            sd = -1.0 if r < 2 else 1.0  # 4-row diff pattern
            row = 4 * m + r
            w2neg[row, m] = -sd          # HH2
            w2pos[row, m] = sd
            w2neg[row, 32 + m] = -1.0    # HL2
            w2pos[row, 32 + m] = 1.0
            w2neg[row, 64 + m] = sd      # LH2
            w2pos[row, 64 + m] = sd
    return w1q, w1, w2neg, w2pos


def _build_bass():
    from contextlib import ExitStack

    import concourse.bacc as bacc
    import concourse.bass as bass
    import concourse.mybir as mybir
    import concourse.tile as tile

    F32 = mybir.dt.float32
    BF16 = mybir.dt.bfloat16
    X = mybir.AxisListType.X
    ADD = mybir.AluOpType.add
    ABS = mybir.ActivationFunctionType.Abs

    nc = bacc.Bacc("TRN2", target_bir_lowering=False, debug=False,
                   num_devices=N_CORES)
    ot_d = nc.dram_tensor("ot", [128, OT_COLS], BF16,
                          kind="ExternalInput").ap()
    w_d = nc.dram_tensor("w", [128, 512], BF16, kind="ExternalInput").ap()
    res_d = nc.dram_tensor("res", [128, 4], F32, kind="ExternalOutput").ap()

    with tile.TileContext(nc) as tc, ExitStack() as ctx:
        consts = ctx.enter_context(tc.tile_pool(name="consts", bufs=1))
        dpool = ctx.enter_context(tc.tile_pool(name="dpool", bufs=2))
        cpool = ctx.enter_context(tc.tile_pool(name="cpool", bufs=2))
        psU = ctx.enter_context(tc.tile_pool(name="psU", bufs=2,
                                             space="PSUM"))

        ot_sb = consts.tile([128, OT_COLS], BF16)
        w_all = consts.tile([128, 512], BF16)
        w1q_t = w_all[:, 0:128]
        w1_t = w_all[:, 128:256]
        w2n_t = w_all[:, 256:384]
        w2p_t = w_all[:, 384:512]

        acc = consts.tile([128, 32], F32)
        res_t = consts.tile([128, 4], F32)
        nc.vector.memset(acc[:], 0.0)
        nc.vector.memset(res_t[:], 0.0)

        mm = nc.tensor.matmul

        # Warm the PE to full p-state during the framework preamble.
        warm = consts.tile([128, 512], BF16)
        nc.vector.memset(warm[:], 0.0)
        pswarm = psU.tile([128, 1920], F32, tag="u")
        for _ in range(6):
            mm(pswarm[:, 0:512], lhsT=warm[:, 0:128], rhs=warm[:],
               start=True, stop=True)

        def act_abs(out, in_, accum_out):
            # Abs activation with an IMMEDIATE zero bias (bass converts
            # float biases of non-Copy funcs to an SBUF const AP, which
            # costs ~185 ns extra init per activation).
            imm = lambda v: mybir.ImmediateValue(dtype=F32, value=v)
            eng = nc.scalar
            return eng.add_instruction(mybir.InstActivation(
                name=nc.get_next_instruction_name(),
                func=ABS,
                ins=[eng.lower_ap(in_), imm(0.0), imm(1.0), imm(0.0)],
                outs=[eng.lower_ap(out), eng.lower_ap(accum_out)]))

        # ---- DMA: o-chunks on the Sync HWDGE queue, t-chunks on the
        # Scalar HWDGE queue (SWDGE data loads stall: the Q7 descriptor
        # generator shares SBUF ports with the DVE's 2-port ops).
        # Weights ride the otherwise-idle GpSimd SWDGE queue.
        nc.gpsimd.dma_start(w_all[:], w_d)
        for g in range(N_G):
            c0 = g * G_BLK * COLS
            cw = G_BLK * COLS
            src_o = bass.AP(tensor=ot_d.tensor, offset=c0,
                            ap=[[OT_COLS, 128], [1, cw]])
            src_t = bass.AP(tensor=ot_d.tensor, offset=TCOLS + c0,
                            ap=[[OT_COLS, 128], [1, cw]])
            nc.sync.dma_start(ot_sb[:, c0:c0 + cw], src_o)
            nc.scalar.dma_start(ot_sb[:, TCOLS + c0:TCOLS + c0 + cw], src_t)

        csd_tiles = [None] * N_G
        n_acc = [0]

        def next_acc():
            c = n_acc[0]
            n_acc[0] += 1
            return acc[:, c:c + 1]

        def emit_dve_group(g):
            c0 = g * G_BLK * COLS
            cw = G_BLK * COLS
            d = dpool.tile([128, cw], BF16, tag="d")
            d3 = d[:].rearrange("p (b c) -> p b c", b=G_BLK)
            o3 = ot_sb[:, c0:c0 + cw].rearrange("p (b c) -> p b c", b=G_BLK)
            t3 = ot_sb[:, TCOLS + c0:TCOLS + c0 + cw].rearrange(
                "p (b c) -> p b c", b=G_BLK)
            nc.vector.tensor_sub(d3, o3, t3)

            csd = cpool.tile([128, cw], BF16, tag="csd")
            half = cw // 2
            cs3 = csd[:, 0:half].rearrange("p (b c) -> p b c", b=G_BLK)
            cd3 = csd[:, half:cw].rearrange("p (b c) -> p b c", b=G_BLK)
            de = d3[:, :, 0:COLS // 2]
            do = d3[:, :, COLS // 2:COLS]
            nc.vector.tensor_add(cs3, de, do)
            nc.vector.tensor_sub(cd3, do, de)
            csd_tiles[g] = csd

        def cs_view(b, nb):
            # cs cols for blocks [b, b+nb): [128, 256*nb] contiguous
            g, j = b // G_BLK, b % G_BLK
            return csd_tiles[g][:, 256 * j:256 * (j + nb)]

        def cd_view(b, nb):
            g, j = b // G_BLK, b % G_BLK
            off = G_BLK * 256
            return csd_tiles[g][:, off + 256 * j:off + 256 * (j + nb)]

        def cs_parity(b, nb, odd):
            # cs_e / cs_o over blocks [b, b+nb): [128, nb, 128]
            v = cs_view(b, nb).rearrange("p (b c) -> p b c", b=nb)
            return v[:, :, 128:256] if odd else v[:, :, 0:128]

        for u in range(N_U):
            b0 = u * U_BLK
            g = b0 // G_BLK
            if csd_tiles[g] is None:
                emit_dve_group(g)

            # unit psum [128, 1920]:
            # [S(b0,b0+1) 512 | D(b0,b0+1) 512 | S(b0+2) 256 |
            #  D(b0+2) 256 | L2(b0..b0+2) 384] -- no matmul output
            # crosses a psum bank boundary.
            uT = psU.tile([128, 1920], F32, tag="u")
            mm(uT[:, 0:512], lhsT=w1q_t, rhs=cs_view(b0, 2),
               start=True, stop=True)
            mm(uT[:, 1024:1280], lhsT=w1q_t, rhs=cs_view(b0 + 2, 1),
               start=True, stop=True)
            mm(uT[:, 512:1024], lhsT=w1_t, rhs=cd_view(b0, 2),
               start=True, stop=True)
            mm(uT[:, 1280:1536], lhsT=w1_t, rhs=cd_view(b0 + 2, 1),
               start=True, stop=True)
            mm(uT[:, 1536:1920], lhsT=w2n_t,
               rhs=cs_parity(b0, 3, odd=False), start=True, stop=False)
            mm(uT[:, 1536:1920], lhsT=w2p_t,
               rhs=cs_parity(b0, 3, odd=True), start=False, stop=True)

            act_abs(uT[:], uT[:], next_acc())

        res_col = res_t[:, 0:1]
        nc.vector.tensor_reduce(res_col, acc[:, 0:n_acc[0]], axis=X, op=ADD)
        nc.sync.dma_start(res_d, res_t[:])

    nc.compile()
    return nc


def _get_bass():
    if "nc" not in _CACHE:
        _CACHE["nc"] = _build_bass()
    return _CACHE["nc"]


def _numpy_reference(output, target):
    """Full-precision fallback (only for the never-hit mixed-normalize case)."""
    o = output.astype(np.float64)
    t = target.astype(np.float64)
    if o.min() < 0:
        o = (o + 1.0) * 0.5
    if t.min() < 0:
        t = (t + 1.0) * 0.5

    def dwt(x):
        a = x[:, :, 0::2, 0::2]
        b = x[:, :, 0::2, 1::2]
        c = x[:, :, 1::2, 0::2]
        d = x[:, :, 1::2, 1::2]
        return (0.5 * (a + b + c + d), 0.5 * (-a - b + c + d),
                0.5 * (-a + b - c + d), 0.5 * (a - b - c + d))

    ll_o, lh_o, hl_o, hh_o = dwt(o)
    ll_t, lh_t, hl_t, hh_t = dwt(t)
    tot = (np.abs(lh_o - lh_t).mean() + np.abs(hl_o - hl_t).mean()
           + np.abs(hh_o - hh_t).mean() + 0.1 * np.abs(ll_o - ll_t).mean())
    _, lh2_o, hl2_o, hh2_o = dwt(ll_o)
    _, lh2_t, hl2_t, hh2_t = dwt(ll_t)
    tot += 0.5 * (np.abs(lh2_o - lh2_t).mean() + np.abs(hl2_o - hl2_t).mean()
                  + np.abs(hh2_o - hh2_t).mean())
    return np.float32(tot)


_PERM = np.concatenate([np.arange(0, 512, 4), np.arange(2, 512, 4),
                        np.arange(1, 512, 4), np.arange(3, 512, 4)])


def _run_device(o, t, trace=False):
    """Shard [32,3,512,512] f32 arrays over 8 cores and run the Bass NEFF."""
    import ml_dtypes
    from concourse.bass_utils import run_bass_kernel_spmd

    nc = _get_bass()
    wcat = np.concatenate(_make_weights(), axis=1)

    in_maps = []
    for c in range(N_CORES):
        sl = slice(c * B_PER_CORE, (c + 1) * B_PER_CORE)
        otb = np.empty((128, OT_COLS), ml_dtypes.bfloat16)
        for i, x in enumerate((o, t)):
            xb = x[sl].reshape(ROWS, COLS)[:, _PERM].astype(ml_dtypes.bfloat16)
            otb[:, i * TCOLS:(i + 1) * TCOLS] = (
                xb.reshape(NBLK, 128, COLS).transpose(1, 0, 2)
                .reshape(128, TCOLS))
        in_maps.append({"ot": otb, "w": wcat})
    res = run_bass_kernel_spmd(nc, in_maps, core_ids=list(range(N_CORES)),
                               trace=trace)
    _CACHE["last_result"] = res
    return res


def combine(results, both_norm=True):
    """Combine per-core [128, 4] abs-sum tensors into the scalar loss."""
    m = 0.0
    for r in results:
        m += r.astype(np.float64)[:, 0].sum()
    n1 = float(B * C * (H // 2) * (W // 2))
    scale = 4.0 * n1 if both_norm else 2.0 * n1
    return np.float32(m / scale)


def kernel(output, target):
    o = np.ascontiguousarray(np.asarray(output, dtype=np.float32))
    t = np.ascontiguousarray(np.asarray(target, dtype=np.float32))
    o_norm = bool(o.min() < 0.0)
    t_norm = bool(t.min() < 0.0)
    if o_norm != t_norm:
        # Normalization applied to only one input: the difference is no
        # longer a pure scale of o - t.  Practically unreachable for the
        # randn inputs this problem uses.
        return _numpy_reference(o, t)

    results = [r["res"] for r in _run_device(o, t).results]
    return combine(results, both_norm=o_norm)


# revision 13
# speedup vs baseline: 1.3542x; 1.1387x over previous
"""Trainium2 Bass kernel for the two-level Haar-DWT detail (L1) loss.

Strategy (pure data parallel over batch, 8 NeuronCores):
  - Host casts both inputs to bf16 and permutes each image row's 512
    columns into mod-4 parity groups [c%4==0 | c%4==2 | c%4==1 | c%4==3],
    so the level-1 even/odd halves AND the level-2 parity quarters are
    all contiguous slices on chip.  Per-core data is laid out as
    [128, 48*512] per tensor (partition p = row p of each 128-row
    block), giving the DMA 1.5 MiB transfers with 12 KiB contiguous
    per-partition runs.
  - The full 12.6 MB/core bf16 stream is HBM-bound (~358 GB/s/core);
    compute is arranged so every engine stays under that ~35 us:
      * o-chunks stream on the Sync HWDGE queue, t-chunks on the GpSimd
        SWDGE queue (two queues overlap; the ACT queue stays clean).
      * DVE: d = o - t, then level-1 column combines cs/cd, as three
        packed-bf16 tensor_tensor ops per 12-block group (2x mode).
      * PE: all row combines, in self-contained 3-block psum units
        [S(b,b+1) | D(b,b+1) | S(b+2) | D(b+2) | L2(3 blocks)] =
        [128, 1920] so a unit never crosses a psum bank inside one
        matmul; two units in flight (double buffered).
      * ScalarE: ONE fused Abs-activation + accum_out per unit
        (immediate zero bias to skip the SBUF const read).
  - All band values share one global 1/(4*n1) divisor (LL1's 0.1 and
    level-2's 0.5 weights are baked into the matmul weights), so the
    per-partition accumulator columns are just summed at the end; the
    host combines the 8 [128,4] results in float64.
"""

import numpy as np

B, C, H, W = 32, 3, 512, 512
N_CORES = 8
B_PER_CORE = B // N_CORES
ROWS = B_PER_CORE * C * H          # 6144 image rows per core
COLS = W                           # 512
NBLK = ROWS // 128                 # 48 blocks of [128, 512]
TCOLS = NBLK * COLS                # 24576 cols per tensor in SBUF layout
OT_COLS = 2 * TCOLS                # o | t side by side

G_BLK = 6                          # blocks per DMA/DVE group
N_G = NBLK // G_BLK                # 8 groups
U_BLK = 3                          # blocks per psum unit
N_U = NBLK // U_BLK                # 16 units

_CACHE = {}


def _make_weights():
    import ml_dtypes
    q = ml_dtypes.bfloat16(0.1)  # LL1 loss weight, baked into W1q
    # w1q[k, m]: row pair-combine for the S (col-sum) path.
    # m<64: +q at rows 2m, 2m+1 (pair sum -> LL1, pre-weighted);
    # m=64+mm: -1/+1 (pair diff -> LH1).
    w1q = np.zeros((128, 128), ml_dtypes.bfloat16)
    w1 = np.zeros((128, 128), ml_dtypes.bfloat16)
    for m in range(64):
        w1q[2 * m, m] = q
        w1q[2 * m + 1, m] = q
        w1q[2 * m, 64 + m] = -1.0
        w1q[2 * m + 1, 64 + m] = 1.0
        # plain +-1 for the D (col-diff) path: HL1 | HH1
        w1[2 * m, m] = 1.0
        w1[2 * m + 1, m] = 1.0
        w1[2 * m, 64 + m] = -1.0
        w1[2 * m + 1, 64 + m] = 1.0
    # Level 2 in two accumulating matmuls: psum2 = w2neg @ cs_e +
    # w2pos @ cs_o (cs_e/cs_o = even/odd level-1 column pairs, stored
    # contiguously by the host's parity permutation).  Rows: [HH2 (diff
    # of cd2); HL2 (sum of cd2); LH2 (diff of cs2); 0], with
    # cd2 = cs_o - cs_e, cs2 = cs_e + cs_o.  Zero-padded to 128 outputs
    # so psum rows 96:128 are exact zeros.
    w2neg = np.zeros((128, 128), ml_dtypes.bfloat16)
    w2pos = np.zeros((128, 128), ml_dtypes.bfloat16)
    for m in range(32):
        for r in range(4):
            sd = -1.0 if r < 2 else 1.0  # 4-row diff pattern
            row = 4 * m + r
            w2neg[row, m] = -sd          # HH2
            w2pos[row, m] = sd
            w2neg[row, 32 + m] = -1.0    # HL2
            w2pos[row, 32 + m] = 1.0
            w2neg[row, 64 + m] = sd      # LH2
            w2pos[row, 64 + m] = sd
    return w1q, w1, w2neg, w2pos


def _build_bass():
    from contextlib import ExitStack

    import concourse.bacc as bacc
    import concourse.bass as bass
    import concourse.mybir as mybir
    import concourse.tile as tile

    F32 = mybir.dt.float32
    BF16 = mybir.dt.bfloat16
    X = mybir.AxisListType.X
    ADD = mybir.AluOpType.add
    ABS = mybir.ActivationFunctionType.Abs

    nc = bacc.Bacc("TRN2", target_bir_lowering=False, debug=False,
                   num_devices=N_CORES)
    ot_d = nc.dram_tensor("ot", [128, OT_COLS], BF16,
                          kind="ExternalInput").ap()
    w_d = nc.dram_tensor("w", [128, 512], BF16, kind="ExternalInput").ap()
    res_d = nc.dram_tensor("res", [128, 4], F32, kind="ExternalOutput").ap()

    with tile.TileContext(nc) as tc, ExitStack() as ctx:
        consts = ctx.enter_context(tc.tile_pool(name="consts", bufs=1))
        dpool = ctx.enter_context(tc.tile_pool(name="dpool", bufs=2))
        cpool = ctx.enter_context(tc.tile_pool(name="cpool", bufs=2))
        psU = ctx.enter_context(tc.tile_pool(name="psU", bufs=2,
                                             space="PSUM"))

        ot_sb = consts.tile([128, OT_COLS], BF16)
        w_all = consts.tile([128, 512], BF16)
        w1q_t = w_all[:, 0:128]
        w1_t = w_all[:, 128:256]
        w2n_t = w_all[:, 256:384]
        w2p_t = w_all[:, 384:512]

        acc = consts.tile([128, 32], F32)
        res_t = consts.tile([128, 4], F32)
        nc.vector.memset(acc[:], 0.0)
        nc.vector.memset(res_t[:], 0.0)

        mm = nc.tensor.matmul

        # Warm the PE to full p-state during the framework preamble.
        warm = consts.tile([128, 512], BF16)
        nc.vector.memset(warm[:], 0.0)
        pswarm = psU.tile([128, 1920], F32, tag="u")
        for _ in range(6):
            mm(pswarm[:, 0:512], lhsT=warm[:, 0:128], rhs=warm[:],
               start=True, stop=True)

        def act_abs(out, in_, accum_out):
            # Abs activation with an IMMEDIATE zero bias (bass converts
            # float biases of non-Copy funcs to an SBUF const AP, which
            # costs ~185 ns extra init per activation).
            imm = lambda v: mybir.ImmediateValue(dtype=F32, value=v)
            eng = nc.scalar
            return eng.add_instruction(mybir.InstActivation(
                name=nc.get_next_instruction_name(),
                func=ABS,
                ins=[eng.lower_ap(in_), imm(0.0), imm(1.0), imm(0.0)],
                outs=[eng.lower_ap(out), eng.lower_ap(accum_out)]))

        # ---- DMA: o-chunks on the Sync HWDGE queue, t-chunks on the
        # Scalar HWDGE queue (SWDGE data loads stall: the Q7 descriptor
        # generator shares SBUF ports with the DVE's 2-port ops).
        # Weights ride the otherwise-idle GpSimd SWDGE queue.
        nc.gpsimd.dma_start(w_all[:], w_d)
        for g in range(N_G):
            c0 = g * G_BLK * COLS
            cw = G_BLK * COLS
            src_o = bass.AP(tensor=ot_d.tensor, offset=c0,
                            ap=[[OT_COLS, 128], [1, cw]])
            src_t = bass.AP(tensor=ot_d.tensor, offset=TCOLS + c0,
                            ap=[[OT_COLS, 128], [1, cw]])
            # alternate which queue carries o vs t so both queues see
            # the same byte stream and chunk g's pair lands together
            qa, qb = (nc.sync, nc.scalar) if g % 2 == 0 else \
                     (nc.scalar, nc.sync)
            qa.dma_start(ot_sb[:, c0:c0 + cw], src_o)
            qb.dma_start(ot_sb[:, TCOLS + c0:TCOLS + c0 + cw], src_t)

        csd_tiles = [None] * N_G
        n_acc = [0]

        def next_acc():
            c = n_acc[0]
            n_acc[0] += 1
            return acc[:, c:c + 1]

        def emit_dve_group(g):
            c0 = g * G_BLK * COLS
            cw = G_BLK * COLS
            d = dpool.tile([128, cw], BF16, tag="d")
            d3 = d[:].rearrange("p (b c) -> p b c", b=G_BLK)
            o3 = ot_sb[:, c0:c0 + cw].rearrange("p (b c) -> p b c", b=G_BLK)
            t3 = ot_sb[:, TCOLS + c0:TCOLS + c0 + cw].rearrange(
                "p (b c) -> p b c", b=G_BLK)
            nc.vector.tensor_sub(d3, o3, t3)

            csd = cpool.tile([128, cw], BF16, tag="csd")
            half = cw // 2
            cs3 = csd[:, 0:half].rearrange("p (b c) -> p b c", b=G_BLK)
            cd3 = csd[:, half:cw].rearrange("p (b c) -> p b c", b=G_BLK)
            de = d3[:, :, 0:COLS // 2]
            do = d3[:, :, COLS // 2:COLS]
            nc.vector.tensor_add(cs3, de, do)
            nc.vector.tensor_sub(cd3, do, de)
            csd_tiles[g] = csd

        def cs_view(b, nb):
            # cs cols for blocks [b, b+nb): [128, 256*nb] contiguous
            g, j = b // G_BLK, b % G_BLK
            return csd_tiles[g][:, 256 * j:256 * (j + nb)]

        def cd_view(b, nb):
            g, j = b // G_BLK, b % G_BLK
            off = G_BLK * 256
            return csd_tiles[g][:, off + 256 * j:off + 256 * (j + nb)]

        def cs_parity(b, nb, odd):
            # cs_e / cs_o over blocks [b, b+nb): [128, nb, 128]
            v = cs_view(b, nb).rearrange("p (b c) -> p b c", b=nb)
            return v[:, :, 128:256] if odd else v[:, :, 0:128]

        for u in range(N_U):
            b0 = u * U_BLK
            g = b0 // G_BLK
            if csd_tiles[g] is None:
                emit_dve_group(g)

            # unit psum [128, 1920]:
            # [S(b0,b0+1) 512 | D(b0,b0+1) 512 | S(b0+2) 256 |
            #  D(b0+2) 256 | L2(b0..b0+2) 384] -- no matmul output
            # crosses a psum bank boundary.
            uT = psU.tile([128, 1920], F32, tag="u")
            mm(uT[:, 0:512], lhsT=w1q_t, rhs=cs_view(b0, 2),
               start=True, stop=True)
            mm(uT[:, 1024:1280], lhsT=w1q_t, rhs=cs_view(b0 + 2, 1),
               start=True, stop=True)
            mm(uT[:, 512:1024], lhsT=w1_t, rhs=cd_view(b0, 2),
               start=True, stop=True)
            mm(uT[:, 1280:1536], lhsT=w1_t, rhs=cd_view(b0 + 2, 1),
               start=True, stop=True)
            mm(uT[:, 1536:1920], lhsT=w2n_t,
               rhs=cs_parity(b0, 3, odd=False), start=True, stop=False)
            mm(uT[:, 1536:1920], lhsT=w2p_t,
               rhs=cs_parity(b0, 3, odd=True), start=False, stop=True)

            act_abs(uT[:], uT[:], next_acc())

        res_col = res_t[:, 0:1]
        nc.vector.tensor_reduce(res_col, acc[:, 0:n_acc[0]], axis=X, op=ADD)
        nc.sync.dma_start(res_d, res_t[:])

    nc.compile()
    return nc


def _get_bass():
    if "nc" not in _CACHE:
        _CACHE["nc"] = _build_bass()
    return _CACHE["nc"]


def _numpy_reference(output, target):
    """Full-precision fallback (only for the never-hit mixed-normalize case)."""
    o = output.astype(np.float64)
    t = target.astype(np.float64)
    if o.min() < 0:
        o = (o + 1.0) * 0.5
    if t.min() < 0:
        t = (t + 1.0) * 0.5

    def dwt(x):
        a = x[:, :, 0::2, 0::2]
        b = x[:, :, 0::2, 1::2]
        c = x[:, :, 1::2, 0::2]
        d = x[:, :, 1::2, 1::2]
        return (0.5 * (a + b + c + d), 0.5 * (-a - b + c + d),
                0.5 * (-a + b - c + d), 0.5 * (a - b - c + d))

    ll_o, lh_o, hl_o, hh_o = dwt(o)
    ll_t, lh_t, hl_t, hh_t = dwt(t)
    tot = (np.abs(lh_o - lh_t).mean() + np.abs(hl_o - hl_t).mean()
           + np.abs(hh_o - hh_t).mean() + 0.1 * np.abs(ll_o - ll_t).mean())
    _, lh2_o, hl2_o, hh2_o = dwt(ll_o)
    _, lh2_t, hl2_t, hh2_t = dwt(ll_t)
    tot += 0.5 * (np.abs(lh2_o - lh2_t).mean() + np.abs(hl2_o - hl2_t).mean()
                  + np.abs(hh2_o - hh2_t).mean())
    return np.float32(tot)


_PERM = np.concatenate([np.arange(0, 512, 4), np.arange(2, 512, 4),
                        np.arange(1, 512, 4), np.arange(3, 512, 4)])


def _run_device(o, t, trace=False):
    """Shard [32,3,512,512] f32 arrays over 8 cores and run the Bass NEFF."""
    import ml_dtypes
    from concourse.bass_utils import run_bass_kernel_spmd

    nc = _get_bass()
    wcat = np.concatenate(_make_weights(), axis=1)

    in_maps = []
    for c in range(N_CORES):
        sl = slice(c * B_PER_CORE, (c + 1) * B_PER_CORE)
        otb = np.empty((128, OT_COLS), ml_dtypes.bfloat16)
        for i, x in enumerate((o, t)):
            xb = x[sl].reshape(ROWS, COLS)[:, _PERM].astype(ml_dtypes.bfloat16)
            otb[:, i * TCOLS:(i + 1) * TCOLS] = (
                xb.reshape(NBLK, 128, COLS).transpose(1, 0, 2)
                .reshape(128, TCOLS))
        in_maps.append({"ot": otb, "w": wcat})
    res = run_bass_kernel_spmd(nc, in_maps, core_ids=list(range(N_CORES)),
                               trace=trace)
    _CACHE["last_result"] = res
    return res


def combine(results, both_norm=True):
    """Combine per-core [128, 4] abs-sum tensors into the scalar loss."""
    m = 0.0
    for r in results:
        m += r.astype(np.float64)[:, 0].sum()
    n1 = float(B * C * (H // 2) * (W // 2))
    scale = 4.0 * n1 if both_norm else 2.0 * n1
    return np.float32(m / scale)


def kernel(output, target):
    o = np.ascontiguousarray(np.asarray(output, dtype=np.float32))
    t = np.ascontiguousarray(np.asarray(target, dtype=np.float32))
    o_norm = bool(o.min() < 0.0)
    t_norm = bool(t.min() < 0.0)
    if o_norm != t_norm:
        # Normalization applied to only one input: the difference is no
        # longer a pure scale of o - t.  Practically unreachable for the
        # randn inputs this problem uses.
        return _numpy_reference(o, t)

    results = [r["res"] for r in _run_device(o, t).results]
    return combine(results, both_norm=o_norm)
